# revision 35
# baseline (speedup 1.0000x reference)
"""Multi-head attention forward on 8 TRN2 NeuronCores.

Problem: B=4, S=2048, D=1024, H=16, d_k=64, fp32 in/out, mask == all-ones
(per the input spec the mask is always ones, so masking is a no-op and is
skipped).

Sharding (data-parallel over batch x query-blocks, no collectives):
  core c -> batch b = c//2, query rows [ (c%2)*1024, (c%2)*1024+1024 ).
Each core computes the full forward for its 1024 query rows: Q/K/V
projections (K/V over all 2048 keys of its batch), attention, and the
output projection. The host pre-transposes inputs (pure data movement) and
concatenates the 8 per-core outputs. Every FLOP runs on-device.

Device algorithm per core (layouts chosen so no on-device transposes are
needed):
  A) QhT[(h,dk), q]  = wq.T @ Q.T   (d_model on partitions, fp32r matmuls)
  B) KhT[(h,dk), k]  = wk.T @ K.T
  C) Vh[k, (h,dk)]   = (V.T).T @ wv, stored bf16 with a ones column per head
  D) attention, software-pipelined so the ACT engine (exp) never idles:
     queries processed in 512-wide halves; per (pair, half, k-tile) one
     [128,1024] PSUM score tile holds both heads of the pair (head A cols
     0:512, head B cols 512:1024, disjoint PE row groups -> the two score
     matmuls run concurrently); one ACTIVATE(exp, scale=1/8) covers both;
     attn_unnorm^T accumulates via [Vh | 1]^T @ exp -- the ones column makes
     PSUM row 64 the softmax denominator l. The next k-tile's score matmuls
     are emitted BEFORE this tile's attn@V matmuls, so the PE fills ACT's
     shadow and ACT stays ~100% busy. Denominators are inverted with the
     ~5x-faster reciprocal_approx_fast per pair, broadcast on GpSimd, and
     multiplied in off the critical path.
  E) out[q, d] = attnT.T @ wo + bias (accumulate over the (h,dk) axis;
     pair 7 is the last contraction step so E overlaps pair 7's normalize).
"""

import os
import sys

for _p in ("/root/.axon_site/_ro/trn_rl_repo", "/opt/trn_rl_repo"):
    if os.path.isdir(_p) and _p not in sys.path:
        sys.path.append(_p)

import ml_dtypes
import numpy as np

import concourse.bass as bass  # noqa: F401  (import keeps bass_rust registered)
import concourse.tile as tile
from concourse import bacc, mybir
from concourse.bass_utils import run_bass_kernel_spmd

P = 128
D = 1024  # d_model
S = 2048  # sequence length (keys per batch)
QL = 1024  # query rows per core
H = 16
DK = 64
NPAIR = H // 2  # pair p holds head 2p on partitions 0-63, head 2p+1 on 64-127
DKT = D // P  # 8 contraction tiles over d_model
KMT = S // P  # 16 key-row tiles
QMT = QL // P  # 8 query-row tiles
F32 = mybir.dt.float32
F32R = mybir.dt.float32r
BF16 = mybir.dt.bfloat16
EXP = mybir.ActivationFunctionType.Exp
I16 = mybir.dt.int16
# exp via Schraudolph bit-trick on the DVE for a subset of k-tiles, so the
# softmax splits across two engines: y = bitcast_bf16(int16(x*C1 + C2)).
# C1 folds the 1/sqrt(d_k)=1/8 score scale; C=5.59 minimizes max rel err
# (~3%); numpy end-to-end check: adds ~4e-3 to the final rel error.
SCH_C1 = float((2 ** 7 / np.log(2)) / 8.0)
SCH_C2 = float(127 * 2 ** 7 - 5.59)
DVE_K = (1, 4, 7, 10, 13)  # k-tiles whose exp runs on the DVE

LAST_RESULTS = None  # test harness reads exec_time_ns from here


def _r(ap):
    """Reinterpret an fp32 AP as float32r (FP22-truncated matmul, full PE rate)."""
    return ap.bitcast(F32R)


def _build_nc():
    nc = bacc.Bacc("TRN2", debug=False, target_bir_lowering=False)

    qt = nc.dram_tensor("qt", [D, QL], F32, kind="ExternalInput").ap()
    ktd = nc.dram_tensor("ktd", [D, S], F32, kind="ExternalInput").ap()
    vtd = nc.dram_tensor("vtd", [S, D], BF16, kind="ExternalInput").ap()
    wq = nc.dram_tensor("wq", [D, D], F32, kind="ExternalInput").ap()
    wk = nc.dram_tensor("wk", [D, D], F32, kind="ExternalInput").ap()
    wv = nc.dram_tensor("wv", [D, D], BF16, kind="ExternalInput").ap()
    wo = nc.dram_tensor("wo", [D, D], BF16, kind="ExternalInput").ap()
    wob = nc.dram_tensor("wob", [1, D], F32, kind="ExternalInput").ap()
    out = nc.dram_tensor("out", [QL, D], F32, kind="ExternalOutput").ap()

    qt3 = qt.rearrange("(kt p) q -> p kt q", p=P)  # [128, 8, 1024]
    kt3 = ktd.rearrange("(kt p) s -> p kt s", p=P)  # [128, 8, 2048]
    # wq/wk/vtd arrive host-pre-tiled so each device tile is one contiguous
    # 512KB/256KB read (strided weight-tile DMAs were 4-5x slower and
    # stalled the projection matmul stream)
    vt4 = vtd.rearrange("(km p) (kt s) -> p km kt s", p=P, s=P)
    wq3 = wq.rearrange("(kt p) e -> p kt e", p=P)
    wk3 = wk.rearrange("(kt p) e -> p kt e", p=P)
    wv3 = wv.rearrange("(kt p) e -> p kt e", p=P)
    wo3 = wo.rearrange("(kt p) e -> p kt e", p=P)
    out3 = out.rearrange("(mt p) e -> p mt e", p=P)

    with tile.TileContext(nc) as tc:
        mm = nc.tensor.matmul

        # ------- persistent SBUF (left stack base) -------
        pers = tc.alloc_tile_pool(name="pers", bufs=1)
        qh = pers.tile([P, NPAIR, QL], F32R)  # QhT: pair partitions x pair x q
        kh = pers.tile([P, NPAIR, S], F32R)  # KhT
        bias_sb = pers.tile([P, D], F32)

        # bias broadcast [1,D] -> [128,D] via 0-stride-partition DMA read
        wob_bcast = bass.AP(tensor=wob.tensor, offset=wob.offset,
                            ap=[[0, P]] + [list(d) for d in wob.ap[1:]])
        nc.gpsimd.dma_start(out=bias_sb, in_=wob_bcast)

        # ------- big staging slot-chains -------
        # chainX (right stack): qt -> kt_lo -> vh reuse one ~33KB slot
        # chainY (left stack):  kt_hi -> wv reuse another
        # Projections run k-outer so both the activations and the weight
        # chunks (all contiguous 512KB reads) are consumed in DMA arrival
        # order; staging is round-robined across the sync/gpsimd queues with
        # weights on the scalar queue, ordered by need-time.
        chX = tc.alloc_tile_pool(name="chX", bufs=1, side="right")
        chY = tc.alloc_tile_pool(name="chY", bufs=1)
        pW = tc.alloc_tile_pool(name="pW", bufs=8)  # wq/wk chunk ring
        pSm = tc.alloc_tile_pool(name="pSm", bufs=2)  # vt tiles

        qt_sb = chX.tile([P, DKT, QL], F32R, tag="big")
        kt_hi = chY.tile([P, DKT, S // 2], F32R, tag="bigY")
        kt_lo = chX.tile([P, DKT, S // 2], F32R, tag="big")
        wq_c = [pW.tile([P, D], F32R, tag="w", name="wqc") for _ in range(DKT)]
        wk_c = [pW.tile([P, D], F32R, tag="w", name="wkc") for _ in range(DKT)]
        qs3 = (nc.sync, nc.gpsimd, nc.scalar)
        rr = [0]

        def stage(out_ap, in_ap):
            qs3[rr[0] % 3].dma_start(out=out_ap, in_=in_ap)
            rr[0] += 1

        # need-order: qt (A), wq (A), kt_hi (B-hi), wk (B), kt_lo (B-lo;
        # these DMAs unblock when A's last read of qt completes)
        for k in range(DKT):
            stage(qt_sb[:, k, :], _r(qt3[:, k, :]))
        for k in range(DKT):
            stage(wq_c[k], _r(wq3[:, k, :]))
        for k in range(DKT):
            stage(kt_hi[:, k, :], _r(kt3[:, k, S // 2 : S]))
        for k in range(DKT):
            stage(wk_c[k], _r(wk3[:, k, :]))
        for k in range(DKT):
            stage(kt_lo[:, k, :], _r(kt3[:, k, 0 : S // 2]))
        # wv takes kt_hi's slot after the hi half of B
        psum_pr = tc.alloc_tile_pool(name="psum_pr", bufs=8, space="PSUM")

        def proj_pass(w_chunks, src, dst_cols, n_mt=DKT):
            # one k-outer projection pass: 8 psum groups, streamed inputs
            pss = [psum_pr.tile([P, 512], F32, tag="prps", name="prp")
                   for _ in range(n_mt)]
            for k in range(DKT):
                st, sp = k == 0, k == DKT - 1
                for mt in range(n_mt):
                    mm(pss[mt], w_chunks[k][:, mt * P : (mt + 1) * P],
                       src[:, k, :], start=st, stop=sp, skip_group_check=True)
            for mt in range(n_mt):
                # alternate engines so bank drain doesn't gate the next pass
                if mt % 2 == 0:
                    nc.scalar.copy(out=dst_cols(mt), in_=pss[mt])
                else:
                    nc.vector.tensor_copy(out=dst_cols(mt), in_=pss[mt])

        # ---------------- phase A: Q projection ----------------
        for qh2 in (0, 1):
            qs = slice(qh2 * 512, qh2 * 512 + 512)
            proj_pass(wq_c, qt_sb[:, :, qs],
                      lambda mt, qs=qs: qh[:, mt, qs])

        # ---------------- phase B: K projection (hi half, then lo) -------
        wv_sb = None
        for half in (1, 0):
            kt_sb = kt_hi if half == 1 else kt_lo
            for qh2 in (0, 1):
                qs = slice(qh2 * 512, qh2 * 512 + 512)
                base = half * (S // 2) + qh2 * 512
                proj_pass(wk_c, kt_sb[:, :, qs],
                          lambda mt, base=base: kh[:, mt, base : base + 512])
            if half == 1:
                # wv takes kt_hi's slot; DMA overlaps the lo-half compute
                wv_sb = chY.tile([P, DKT, D], BF16, tag="bigY")
                nc.gpsimd.dma_start(out=wv_sb, in_=wv3)

        # ---------------- phase C: V projection ----------------
        # vh takes kt_lo's slot (chainX); col 64 of each head group is ones
        vh = chX.tile([P, KMT, H, 66], BF16, tag="big")
        nc.vector.memset(vh[:, :, :, 64:65], 1.0)
        for km in range(KMT):
            vt_t = pSm.tile([P, DKT, P], BF16, tag="smv")
            nc.gpsimd.dma_start(out=vt_t, in_=vt4[:, km, :, :])
            ps0 = psum_pr.tile([P, 512], F32, tag="prps")
            ps1 = psum_pr.tile([P, 512], F32, tag="prps")
            for k in range(DKT):
                st, sp = k == 0, k == DKT - 1
                mm(ps0, vt_t[:, k, :], wv_sb[:, k, 0:512],
                   start=st, stop=sp, skip_group_check=True)
                mm(ps1, vt_t[:, k, :], wv_sb[:, k, 512:1024],
                   start=st, stop=sp, skip_group_check=True)
            nc.scalar.copy(
                out=vh[:, km, 0:8, 0:64],
                in_=ps0.rearrange("p (h e) -> p h e", e=DK),
            )
            nc.vector.tensor_copy(
                out=vh[:, km, 8:16, 0:64],
                in_=ps1.rearrange("p (h e) -> p h e", e=DK),
            )
        pSm.release()
        pW.release()
        psum_pr.release()

        # wo takes chY's slot (free after C); its DMA overlaps phase D
        wo_sb = chY.tile([P, DKT, D], BF16, tag="bigY")
        for k in range(DKT):
            nc.gpsimd.dma_start(out=wo_sb[:, k, :], in_=wo3[:, k, :])

        # ---------------- phase D: attention, ACT-saturated pipeline -----
        pAttn = tc.alloc_tile_pool(name="pAttn", bufs=1)
        attn = pAttn.tile([P, NPAIR, QL], BF16)  # normalized attn^T

        # PSUM budget (8 banks): st [128,1024]x2 bufs = 4, av [65,512]x4 = 4.
        # Every attn@V matmul is deferred one pipeline slot (two for DVE-exp
        # slots), so the PE never blocks on an exp engine mid-FIFO: per slot
        # it runs the next score tile plus the previous slot's attn@V, both
        # of whose inputs are already resident.
        psum_st = tc.alloc_tile_pool(name="psum_st", bufs=3, space="PSUM")
        psum_av = tc.alloc_tile_pool(name="psum_av", bufs=2, space="PSUM")
        pEx = tc.alloc_tile_pool(name="pEx", bufs=4)
        pNr = tc.alloc_tile_pool(name="pNr", bufs=3)  # bcs rotation
        pRv = tc.alloc_tile_pool(name="pRv", bufs=2)  # per-head 1/l rows
        pLl = tc.alloc_tile_pool(name="pLl", bufs=2)  # l-row SBUF staging

        halves = [(p, h) for p in range(NPAIR) for h in range(2)]
        NG = len(halves) * KMT  # 256 global pipeline steps

        def st_tile(g):
            # emit both heads' score matmuls for global step g
            (p, h), k = halves[g // KMT], g % KMT
            q0 = h * 512
            stAB = psum_st.tile([P, 1024], F32, tag="st")
            ksl = kh[:, p, k * P : (k + 1) * P]
            mm(stAB[:, 0:512], ksl[0:64, :], qh[0:64, p, q0 : q0 + 512],
               skip_group_check=True)
            mm(stAB[:, 512:1024], ksl[64:128, :], qh[64:128, p, q0 : q0 + 512],
               skip_group_check=True)
            return stAB

        av_state = {}  # (p, h) -> (avA, avB); rv_state: p -> (rv0, rv1)
        rv_state = {}

        def emit_av(e):
            p, h, k, exq = e["p"], e["h"], e["k"], e["exq"]
            hA, hB = 2 * p, 2 * p + 1
            if k == 0:
                av_state[(p, h)] = (
                    psum_av.tile([65, 512], F32, tag="av", name="avA"),
                    psum_av.tile([65, 512], F32, tag="av", name="avB"))
            avs = av_state[(p, h)]
            stF, spF = k == 0, k == KMT - 1
            mm(avs[0], vh[:, k, hA, 0:65], exq[:, 0:512],
               start=stF, stop=spF, skip_group_check=True)
            mm(avs[1], vh[:, k, hB, 0:65], exq[:, 512:1024],
               start=stF, stop=spF, skip_group_check=True)
            if spF:
                emit_evict(p, h)

        def emit_evict(p, h):
            # rows 0-63 attn_unnorm^T, row 64 l -> 1/l via fast reciprocal
            q0 = h * 512
            avs = av_state.pop((p, h))
            if h == 0:
                rv_state[p] = (pRv.tile([1, QL], F32, tag="rv", name="rv0"),
                               pRv.tile([1, QL], F32, tag="rv", name="rv1"))
            rv_pair = rv_state[p]
            lrows = []
            for head, av in ((0, avs[0]), (1, avs[1])):
                rows = slice(0, 64) if head == 0 else slice(64, 128)
                nc.scalar.copy(out=attn[rows, p, q0 : q0 + 512],
                               in_=av[0:64, :])
                lrow = pLl.tile([1, 512], F32, tag="l")
                nc.vector.tensor_copy(out=lrow, in_=av[64:65, :])
                lrows.append(lrow)
            # only the lrow copies are urgent (they free the av banks);
            # the reciprocal/broadcast/multiply chain is deferred six slots
            # so the next half's DVE-exp isn't queued behind it
            def norm(p=p, q0=q0, rv_pair=rv_pair, lrows=lrows):
                qs = slice(q0, q0 + 512)
                for head in (0, 1):
                    rows = slice(0, 64) if head == 0 else slice(64, 128)
                    nc.vector.reciprocal_approx_fast(
                        out=rv_pair[head][:, qs], in_=lrows[head])
                    bcs = pNr.tile([P, 512], F32, tag="bc", name="bcs")
                    nc.gpsimd.partition_broadcast(bcs, rv_pair[head][:, qs])
                    nc.vector.tensor_mul(attn[rows, p, qs], attn[rows, p, qs],
                                         bcs[rows, :])
            norm_q.append(norm)
            if h == 1:
                rv_state.pop(p)

        st_q = [st_tile(0), st_tile(1)]
        pending = []
        norm_q = []
        for g in range(NG):
            (p, h), k = halves[g // KMT], g % KMT
            cur_st = st_q.pop(0)
            while pending and pending[0]["ready"] <= g:
                emit_av(pending.pop(0))
            if k in DVE_K:
                exf = pEx.tile([P, 1024], I16, tag="ex", bufs=4)
                nc.vector.tensor_scalar(out=exf, in0=cur_st, scalar1=SCH_C1,
                                        scalar2=SCH_C2,
                                        op0=mybir.AluOpType.mult,
                                        op1=mybir.AluOpType.add)
                exq = exf.bitcast(BF16)
                ready = g + 2
            else:
                ex = pEx.tile([P, 1024], BF16, tag="ex", bufs=4)
                nc.scalar.activation(ex, cur_st, EXP, scale=0.125)
                exq = ex
                ready = g + 1
            if g + 2 < NG:
                st_q.append(st_tile(g + 2))
            pending.append({"p": p, "h": h, "k": k, "exq": exq,
                            "ready": ready})
            if k == 6 and norm_q:
                norm_q.pop(0)()
        for e in pending:
            emit_av(e)
        for f in norm_q:
            f()

        pLl.release()
        pRv.release()
        pNr.release()
        pEx.release()
        psum_av.release()
        psum_st.release()
        chX.release()

        # ---------------- phase E: output projection ----------------
        # contraction order 0..7 leaves pair 7 last, so the first seven
        # accumulation steps overlap pair 7's normalize tail.
        psum_E = tc.alloc_tile_pool(name="psum_E", bufs=4, space="PSUM")
        pOut = tc.alloc_tile_pool(name="pOut", bufs=2)

        for mt in range(QMT):
            ps0 = psum_E.tile([P, 512], F32, tag="eps")
            ps1 = psum_E.tile([P, 512], F32, tag="eps")
            for k in range(DKT):
                st, sp = k == 0, k == DKT - 1
                a_sl = attn[:, k, mt * P : (mt + 1) * P]
                mm(ps0, a_sl, wo_sb[:, k, 0:512],
                   start=st, stop=sp, skip_group_check=True)
                mm(ps1, a_sl, wo_sb[:, k, 512:1024],
                   start=st, stop=sp, skip_group_check=True)
            o_sb = pOut.tile([P, D], F32, tag="osb")
            nc.vector.tensor_add(out=o_sb[:, 0:512], in0=ps0,
                                 in1=bias_sb[:, 0:512])
            eng = (nc.sync, nc.gpsimd, nc.scalar)[(2 * mt) % 3]
            eng.dma_start(out=out3[:, mt, 0:512], in_=o_sb[:, 0:512])
            nc.vector.tensor_add(out=o_sb[:, 512:1024], in0=ps1,
                                 in1=bias_sb[:, 512:1024])
            eng = (nc.sync, nc.gpsimd, nc.scalar)[(2 * mt + 1) % 3]
            eng.dma_start(out=out3[:, mt, 512:1024], in_=o_sb[:, 512:1024])

        pOut.release()
        psum_E.release()
        pAttn.release()
        chY.release()
        pers.release()

    nc.compile()
    return nc


_NC = None


def _get_nc():
    global _NC
    if _NC is None:
        _NC = _build_nc()
    return _NC


def kernel(Q, K, V, mask, W_q, W_k, W_v, W_o_w, W_o_b):
    global LAST_RESULTS
    Q = np.asarray(Q, dtype=np.float32)
    K = np.asarray(K, dtype=np.float32)
    V = np.asarray(V, dtype=np.float32)
    W_q = np.asarray(W_q, dtype=np.float32)
    W_k = np.asarray(W_k, dtype=np.float32)
    W_v = np.asarray(W_v, dtype=np.float32)
    W_o_w = np.asarray(W_o_w, dtype=np.float32)
    W_o_b = np.asarray(W_o_b, dtype=np.float32)

    # weight shards (shared by all cores); host-side transpose is data
    # movement only
    wq_h = np.ascontiguousarray(W_q.transpose(1, 0, 2).reshape(D, D))
    wk_h = np.ascontiguousarray(W_k.transpose(1, 0, 2).reshape(D, D))
    wv_bf = np.ascontiguousarray(
        W_v.transpose(1, 0, 2).reshape(D, D).astype(ml_dtypes.bfloat16))
    wo_h = np.ascontiguousarray(W_o_w.T.astype(ml_dtypes.bfloat16))
    wob_h = np.ascontiguousarray(W_o_b.reshape(1, D))

    in_maps = []
    for c in range(8):
        b, qs = c // 2, (c % 2) * QL
        in_maps.append({
            "qt": np.ascontiguousarray(Q[b, qs : qs + QL, :].T),
            "ktd": np.ascontiguousarray(K[b].T),
            "vtd": np.ascontiguousarray(
                V[b].T.reshape(8, P, 16, P).transpose(2, 1, 0, 3)
                .reshape(S, D).astype(ml_dtypes.bfloat16)),
            "wq": wq_h,
            "wk": wk_h,
            "wv": wv_bf,
            "wo": wo_h,
            "wob": wob_h,
        })

    nc = _get_nc()
    res = run_bass_kernel_spmd(nc, in_maps, core_ids=list(range(8)))
    LAST_RESULTS = res

    out = np.empty((4, 2 * QL, D), dtype=np.float32)
    for c in range(8):
        b, qs = c // 2, (c % 2) * QL
        out[b, qs : qs + QL, :] = res.results[c]["out"]
    return out


# revision 36
# speedup vs baseline: 1.0017x; 1.0017x over previous
"""Multi-head attention forward on 8 TRN2 NeuronCores.

Problem: B=4, S=2048, D=1024, H=16, d_k=64, fp32 in/out, mask == all-ones
(per the input spec the mask is always ones, so masking is a no-op and is
skipped).

Sharding (data-parallel over batch x query-blocks, no collectives):
  core c -> batch b = c//2, query rows [ (c%2)*1024, (c%2)*1024+1024 ).
Each core computes the full forward for its 1024 query rows: Q/K/V
projections (K/V over all 2048 keys of its batch), attention, and the
output projection. The host pre-transposes inputs (pure data movement) and
concatenates the 8 per-core outputs. Every FLOP runs on-device.

Device algorithm per core (layouts chosen so no on-device transposes are
needed):
  A) QhT[(h,dk), q]  = wq.T @ Q.T   (d_model on partitions, fp32r matmuls)
  B) KhT[(h,dk), k]  = wk.T @ K.T
  C) Vh[k, (h,dk)]   = (V.T).T @ wv, stored bf16 with a ones column per head
  D) attention, software-pipelined so the ACT engine (exp) never idles:
     queries processed in 512-wide halves; per (pair, half, k-tile) one
     [128,1024] PSUM score tile holds both heads of the pair (head A cols
     0:512, head B cols 512:1024, disjoint PE row groups -> the two score
     matmuls run concurrently); one ACTIVATE(exp, scale=1/8) covers both;
     attn_unnorm^T accumulates via [Vh | 1]^T @ exp -- the ones column makes
     PSUM row 64 the softmax denominator l. The next k-tile's score matmuls
     are emitted BEFORE this tile's attn@V matmuls, so the PE fills ACT's
     shadow and ACT stays ~100% busy. Denominators are inverted with the
     ~5x-faster reciprocal_approx_fast per pair, broadcast on GpSimd, and
     multiplied in off the critical path.
  E) out[q, d] = attnT.T @ wo + bias (accumulate over the (h,dk) axis;
     pair 7 is the last contraction step so E overlaps pair 7's normalize).
"""

import os
import sys

for _p in ("/root/.axon_site/_ro/trn_rl_repo", "/opt/trn_rl_repo"):
    if os.path.isdir(_p) and _p not in sys.path:
        sys.path.append(_p)

import ml_dtypes
import numpy as np

import concourse.bass as bass  # noqa: F401  (import keeps bass_rust registered)
import concourse.tile as tile
from concourse import bacc, mybir
from concourse.bass_utils import run_bass_kernel_spmd

P = 128
D = 1024  # d_model
S = 2048  # sequence length (keys per batch)
QL = 1024  # query rows per core
H = 16
DK = 64
NPAIR = H // 2  # pair p holds head 2p on partitions 0-63, head 2p+1 on 64-127
DKT = D // P  # 8 contraction tiles over d_model
KMT = S // P  # 16 key-row tiles
QMT = QL // P  # 8 query-row tiles
F32 = mybir.dt.float32
F32R = mybir.dt.float32r
BF16 = mybir.dt.bfloat16
EXP = mybir.ActivationFunctionType.Exp
I16 = mybir.dt.int16
# exp via Schraudolph bit-trick on the DVE for a subset of k-tiles, so the
# softmax splits across two engines: y = bitcast_bf16(int16(x*C1 + C2)).
# C1 folds the 1/sqrt(d_k)=1/8 score scale; C=5.59 minimizes max rel err
# (~3%); numpy end-to-end check: adds ~4e-3 to the final rel error.
SCH_C1 = float((2 ** 7 / np.log(2)) / 8.0)
SCH_C2 = float(127 * 2 ** 7 - 5.59)
DVE_K = (1, 4, 7, 10, 13)  # k-tiles whose exp runs on the DVE

LAST_RESULTS = None  # test harness reads exec_time_ns from here


def _r(ap):
    """Reinterpret an fp32 AP as float32r (FP22-truncated matmul, full PE rate)."""
    return ap.bitcast(F32R)


def _build_nc():
    nc = bacc.Bacc("TRN2", debug=False, target_bir_lowering=False)

    qt = nc.dram_tensor("qt", [D, QL], F32, kind="ExternalInput").ap()
    ktd = nc.dram_tensor("ktd", [D, S], F32, kind="ExternalInput").ap()
    vtd = nc.dram_tensor("vtd", [S, D], BF16, kind="ExternalInput").ap()
    wq = nc.dram_tensor("wq", [D, D], F32, kind="ExternalInput").ap()
    wk = nc.dram_tensor("wk", [D, D], F32, kind="ExternalInput").ap()
    wv = nc.dram_tensor("wv", [D, D], BF16, kind="ExternalInput").ap()
    wo = nc.dram_tensor("wo", [D, D], BF16, kind="ExternalInput").ap()
    wob = nc.dram_tensor("wob", [1, D], F32, kind="ExternalInput").ap()
    out = nc.dram_tensor("out", [QL, D], F32, kind="ExternalOutput").ap()

    qt3 = qt.rearrange("(kt p) q -> p kt q", p=P)  # [128, 8, 1024]
    kt3 = ktd.rearrange("(kt p) s -> p kt s", p=P)  # [128, 8, 2048]
    # wq/wk/vtd arrive host-pre-tiled so each device tile is one contiguous
    # 512KB/256KB read (strided weight-tile DMAs were 4-5x slower and
    # stalled the projection matmul stream)
    vt4 = vtd.rearrange("(km p) (kt s) -> p km kt s", p=P, s=P)
    wq3 = wq.rearrange("(kt p) e -> p kt e", p=P)
    wk3 = wk.rearrange("(kt p) e -> p kt e", p=P)
    wv3 = wv.rearrange("(kt p) e -> p kt e", p=P)
    wo3 = wo.rearrange("(kt p) e -> p kt e", p=P)
    out3 = out.rearrange("(mt p) e -> p mt e", p=P)

    with tile.TileContext(nc) as tc:
        mm = nc.tensor.matmul

        # ------- persistent SBUF (left stack base) -------
        pers = tc.alloc_tile_pool(name="pers", bufs=1)
        qh = pers.tile([P, NPAIR, QL], F32R)  # QhT: pair partitions x pair x q
        kh = pers.tile([P, NPAIR, S], F32R)  # KhT
        bias_sb = pers.tile([P, D], F32)

        # bias broadcast [1,D] -> [128,D] via 0-stride-partition DMA read
        wob_bcast = bass.AP(tensor=wob.tensor, offset=wob.offset,
                            ap=[[0, P]] + [list(d) for d in wob.ap[1:]])
        nc.gpsimd.dma_start(out=bias_sb, in_=wob_bcast)

        # ------- big staging slot-chains -------
        # chainX (right stack): qt -> kt_lo -> vh reuse one ~33KB slot
        # chainY (left stack):  kt_hi -> wv reuse another
        # Projections run k-outer so both the activations and the weight
        # chunks (all contiguous 512KB reads) are consumed in DMA arrival
        # order; staging is round-robined across the sync/gpsimd queues with
        # weights on the scalar queue, ordered by need-time.
        chX = tc.alloc_tile_pool(name="chX", bufs=1, side="right")
        chY = tc.alloc_tile_pool(name="chY", bufs=1)
        pW = tc.alloc_tile_pool(name="pW", bufs=8)  # wq/wk chunk ring
        pSm = tc.alloc_tile_pool(name="pSm", bufs=2)  # vt tiles

        qt_sb = chX.tile([P, DKT, QL], F32R, tag="big")
        kt_hi = chY.tile([P, DKT, S // 2], F32R, tag="bigY")
        kt_lo = chX.tile([P, DKT, S // 2], F32R, tag="big")
        wq_c = [pW.tile([P, D], F32R, tag="w", name="wqc") for _ in range(DKT)]
        wk_c = [pW.tile([P, D], F32R, tag="w", name="wkc") for _ in range(DKT)]
        qs3 = (nc.sync, nc.gpsimd, nc.scalar)
        rr = [0]

        def stage(out_ap, in_ap):
            qs3[rr[0] % 3].dma_start(out=out_ap, in_=in_ap)
            rr[0] += 1

        # need-order: qt (A), wq (A), kt_hi (B-hi), wk (B), kt_lo (B-lo;
        # these DMAs unblock when A's last read of qt completes)
        for k in range(DKT):
            stage(qt_sb[:, k, :], _r(qt3[:, k, :]))
        for k in range(DKT):
            stage(wq_c[k], _r(wq3[:, k, :]))
        for k in range(DKT):
            stage(kt_hi[:, k, :], _r(kt3[:, k, S // 2 : S]))
        for k in range(DKT):
            stage(wk_c[k], _r(wk3[:, k, :]))
        for k in range(DKT):
            stage(kt_lo[:, k, :], _r(kt3[:, k, 0 : S // 2]))
        # wv takes kt_hi's slot after the hi half of B
        psum_pr = tc.alloc_tile_pool(name="psum_pr", bufs=8, space="PSUM")

        def proj_pass(w_chunks, src, dst_cols, n_mt=DKT):
            # one k-outer projection pass: 8 psum groups, streamed inputs
            pss = [psum_pr.tile([P, 512], F32, tag="prps", name="prp")
                   for _ in range(n_mt)]
            for k in range(DKT):
                st, sp = k == 0, k == DKT - 1
                for mt in range(n_mt):
                    mm(pss[mt], w_chunks[k][:, mt * P : (mt + 1) * P],
                       src[:, k, :], start=st, stop=sp, skip_group_check=True)
            for mt in range(n_mt):
                # alternate engines so bank drain doesn't gate the next pass
                if mt % 2 == 0:
                    nc.scalar.copy(out=dst_cols(mt), in_=pss[mt])
                else:
                    nc.vector.tensor_copy(out=dst_cols(mt), in_=pss[mt])

        # ---------------- phase A: Q projection ----------------
        for qh2 in (0, 1):
            qs = slice(qh2 * 512, qh2 * 512 + 512)
            proj_pass(wq_c, qt_sb[:, :, qs],
                      lambda mt, qs=qs: qh[:, mt, qs])

        # ---------------- phase B: K projection (hi half, then lo) -------
        wv_sb = None
        for half in (1, 0):
            kt_sb = kt_hi if half == 1 else kt_lo
            for qh2 in (0, 1):
                qs = slice(qh2 * 512, qh2 * 512 + 512)
                base = half * (S // 2) + qh2 * 512
                proj_pass(wk_c, kt_sb[:, :, qs],
                          lambda mt, base=base: kh[:, mt, base : base + 512])
            if half == 1:
                # wv takes kt_hi's slot; DMA overlaps the lo-half compute
                wv_sb = chY.tile([P, DKT, D], BF16, tag="bigY")
                nc.gpsimd.dma_start(out=wv_sb, in_=wv3)

        # ---------------- phase C: V projection ----------------
        # vh takes kt_lo's slot (chainX); col 64 of each head group is ones
        vh = chX.tile([P, KMT, H, 66], BF16, tag="big")
        nc.vector.memset(vh[:, :, :, 64:65], 1.0)
        for km in range(KMT):
            vt_t = pSm.tile([P, DKT, P], BF16, tag="smv")
            nc.gpsimd.dma_start(out=vt_t, in_=vt4[:, km, :, :])
            ps0 = psum_pr.tile([P, 512], F32, tag="prps")
            ps1 = psum_pr.tile([P, 512], F32, tag="prps")
            for k in range(DKT):
                st, sp = k == 0, k == DKT - 1
                mm(ps0, vt_t[:, k, :], wv_sb[:, k, 0:512],
                   start=st, stop=sp, skip_group_check=True)
                mm(ps1, vt_t[:, k, :], wv_sb[:, k, 512:1024],
                   start=st, stop=sp, skip_group_check=True)
            nc.scalar.copy(
                out=vh[:, km, 0:8, 0:64],
                in_=ps0.rearrange("p (h e) -> p h e", e=DK),
            )
            nc.vector.tensor_copy(
                out=vh[:, km, 8:16, 0:64],
                in_=ps1.rearrange("p (h e) -> p h e", e=DK),
            )
        pSm.release()
        pW.release()
        psum_pr.release()

        # wo takes chY's slot (free after C); its DMA overlaps phase D
        wo_sb = chY.tile([P, DKT, D], BF16, tag="bigY")
        for k in range(DKT):
            nc.gpsimd.dma_start(out=wo_sb[:, k, :], in_=wo3[:, k, :])

        # ---------------- phase D: attention, ACT-saturated pipeline -----
        pAttn = tc.alloc_tile_pool(name="pAttn", bufs=1)
        attn = pAttn.tile([P, NPAIR, QL], BF16)  # normalized attn^T

        # PSUM budget (8 banks): st [128,1024]x2 bufs = 4, av [65,512]x4 = 4.
        # Every attn@V matmul is deferred one pipeline slot (two for DVE-exp
        # slots), so the PE never blocks on an exp engine mid-FIFO: per slot
        # it runs the next score tile plus the previous slot's attn@V, both
        # of whose inputs are already resident.
        psum_st = tc.alloc_tile_pool(name="psum_st", bufs=3, space="PSUM")
        psum_av = tc.alloc_tile_pool(name="psum_av", bufs=2, space="PSUM")
        pEx = tc.alloc_tile_pool(name="pEx", bufs=4)
        pNr = tc.alloc_tile_pool(name="pNr", bufs=3)  # bcs rotation
        pRv = tc.alloc_tile_pool(name="pRv", bufs=2)  # per-head 1/l rows
        pLl = tc.alloc_tile_pool(name="pLl", bufs=2)  # l-row SBUF staging

        halves = [(p, h) for p in range(NPAIR) for h in range(2)]
        NG = len(halves) * KMT  # 256 global pipeline steps

        def st_tile(g):
            # emit both heads' score matmuls for global step g
            (p, h), k = halves[g // KMT], g % KMT
            q0 = h * 512
            stAB = psum_st.tile([P, 1024], F32, tag="st")
            ksl = kh[:, p, k * P : (k + 1) * P]
            mm(stAB[:, 0:512], ksl[0:64, :], qh[0:64, p, q0 : q0 + 512],
               skip_group_check=True)
            mm(stAB[:, 512:1024], ksl[64:128, :], qh[64:128, p, q0 : q0 + 512],
               skip_group_check=True)
            return stAB

        av_state = {}  # (p, h) -> (avA, avB); rv_state: p -> (rv0, rv1)
        rv_state = {}

        def emit_av(e):
            p, h, k, exq = e["p"], e["h"], e["k"], e["exq"]
            hA, hB = 2 * p, 2 * p + 1
            if k == 0:
                av_state[(p, h)] = (
                    psum_av.tile([65, 512], F32, tag="av", name="avA"),
                    psum_av.tile([65, 512], F32, tag="av", name="avB"))
            avs = av_state[(p, h)]
            stF, spF = k == 0, k == KMT - 1
            mm(avs[0], vh[:, k, hA, 0:65], exq[:, 0:512],
               start=stF, stop=spF, skip_group_check=True)
            mm(avs[1], vh[:, k, hB, 0:65], exq[:, 512:1024],
               start=stF, stop=spF, skip_group_check=True)
            if spF:
                emit_evict(p, h)

        def emit_evict(p, h):
            # rows 0-63 attn_unnorm^T, row 64 l -> 1/l via fast reciprocal
            q0 = h * 512
            avs = av_state.pop((p, h))
            if h == 0:
                rv_state[p] = (pRv.tile([1, QL], F32, tag="rv", name="rv0"),
                               pRv.tile([1, QL], F32, tag="rv", name="rv1"))
            rv_pair = rv_state[p]
            lrows = []
            for head, av in ((0, avs[0]), (1, avs[1])):
                rows = slice(0, 64) if head == 0 else slice(64, 128)
                nc.scalar.copy(out=attn[rows, p, q0 : q0 + 512],
                               in_=av[0:64, :])
                lrow = pLl.tile([1, 512], F32, tag="l")
                nc.vector.tensor_copy(out=lrow, in_=av[64:65, :])
                lrows.append(lrow)
            qs = slice(q0, q0 + 512)
            for head in (0, 1):
                rows = slice(0, 64) if head == 0 else slice(64, 128)
                nc.vector.reciprocal_approx_fast(
                    out=rv_pair[head][:, qs], in_=lrows[head])
                bcs = pNr.tile([P, 512], F32, tag="bc")
                nc.gpsimd.partition_broadcast(bcs, rv_pair[head][:, qs])
                nc.vector.tensor_mul(attn[rows, p, qs], attn[rows, p, qs],
                                     bcs[rows, :])
            if h == 1:
                rv_state.pop(p)

        st_q = [st_tile(0), st_tile(1)]
        pending = []
        for g in range(NG):
            (p, h), k = halves[g // KMT], g % KMT
            cur_st = st_q.pop(0)
            while pending and pending[0]["ready"] <= g:
                emit_av(pending.pop(0))
            if k in DVE_K:
                exf = pEx.tile([P, 1024], I16, tag="ex", bufs=4)
                nc.vector.tensor_scalar(out=exf, in0=cur_st, scalar1=SCH_C1,
                                        scalar2=SCH_C2,
                                        op0=mybir.AluOpType.mult,
                                        op1=mybir.AluOpType.add)
                exq = exf.bitcast(BF16)
                ready = g + 2
            else:
                ex = pEx.tile([P, 1024], BF16, tag="ex", bufs=4)
                nc.scalar.activation(ex, cur_st, EXP, scale=0.125)
                exq = ex
                ready = g + 1
            if g + 2 < NG:
                st_q.append(st_tile(g + 2))
            pending.append({"p": p, "h": h, "k": k, "exq": exq,
                            "ready": ready})
        for e in pending:
            emit_av(e)

        pLl.release()
        pRv.release()
        pNr.release()
        pEx.release()
        psum_av.release()
        psum_st.release()
        chX.release()

        # ---------------- phase E: output projection ----------------
        # contraction order 0..7 leaves pair 7 last, so the first seven
        # accumulation steps overlap pair 7's normalize tail.
        psum_E = tc.alloc_tile_pool(name="psum_E", bufs=4, space="PSUM")
        pOut = tc.alloc_tile_pool(name="pOut", bufs=2)

        for mt in range(QMT):
            ps0 = psum_E.tile([P, 512], F32, tag="eps")
            ps1 = psum_E.tile([P, 512], F32, tag="eps")
            for k in range(DKT):
                st, sp = k == 0, k == DKT - 1
                a_sl = attn[:, k, mt * P : (mt + 1) * P]
                mm(ps0, a_sl, wo_sb[:, k, 0:512],
                   start=st, stop=sp, skip_group_check=True)
                mm(ps1, a_sl, wo_sb[:, k, 512:1024],
                   start=st, stop=sp, skip_group_check=True)
            o_sb = pOut.tile([P, D], F32, tag="osb")
            nc.vector.tensor_add(out=o_sb[:, 0:512], in0=ps0,
                                 in1=bias_sb[:, 0:512])
            eng = (nc.sync, nc.gpsimd, nc.scalar)[(2 * mt) % 3]
            eng.dma_start(out=out3[:, mt, 0:512], in_=o_sb[:, 0:512])
            nc.vector.tensor_add(out=o_sb[:, 512:1024], in0=ps1,
                                 in1=bias_sb[:, 512:1024])
            eng = (nc.sync, nc.gpsimd, nc.scalar)[(2 * mt + 1) % 3]
            eng.dma_start(out=out3[:, mt, 512:1024], in_=o_sb[:, 512:1024])

        pOut.release()
        psum_E.release()
        pAttn.release()
        chY.release()
        pers.release()

    nc.compile()
    return nc


_NC = None


def _get_nc():
    global _NC
    if _NC is None:
        _NC = _build_nc()
    return _NC


def kernel(Q, K, V, mask, W_q, W_k, W_v, W_o_w, W_o_b):
    global LAST_RESULTS
    Q = np.asarray(Q, dtype=np.float32)
    K = np.asarray(K, dtype=np.float32)
    V = np.asarray(V, dtype=np.float32)
    W_q = np.asarray(W_q, dtype=np.float32)
    W_k = np.asarray(W_k, dtype=np.float32)
    W_v = np.asarray(W_v, dtype=np.float32)
    W_o_w = np.asarray(W_o_w, dtype=np.float32)
    W_o_b = np.asarray(W_o_b, dtype=np.float32)

    # weight shards (shared by all cores); host-side transpose is data
    # movement only
    wq_h = np.ascontiguousarray(W_q.transpose(1, 0, 2).reshape(D, D))
    wk_h = np.ascontiguousarray(W_k.transpose(1, 0, 2).reshape(D, D))
    wv_bf = np.ascontiguousarray(
        W_v.transpose(1, 0, 2).reshape(D, D).astype(ml_dtypes.bfloat16))
    wo_h = np.ascontiguousarray(W_o_w.T.astype(ml_dtypes.bfloat16))
    wob_h = np.ascontiguousarray(W_o_b.reshape(1, D))

    in_maps = []
    for c in range(8):
        b, qs = c // 2, (c % 2) * QL
        in_maps.append({
            "qt": np.ascontiguousarray(Q[b, qs : qs + QL, :].T),
            "ktd": np.ascontiguousarray(K[b].T),
            "vtd": np.ascontiguousarray(
                V[b].T.reshape(8, P, 16, P).transpose(2, 1, 0, 3)
                .reshape(S, D).astype(ml_dtypes.bfloat16)),
            "wq": wq_h,
            "wk": wk_h,
            "wv": wv_bf,
            "wo": wo_h,
            "wob": wob_h,
        })

    nc = _get_nc()
    res = run_bass_kernel_spmd(nc, in_maps, core_ids=list(range(8)))
    LAST_RESULTS = res

    out = np.empty((4, 2 * QL, D), dtype=np.float32)
    for c in range(8):
        b, qs = c // 2, (c % 2) * QL
        out[b, qs : qs + QL, :] = res.results[c]["out"]
    return out
